# revision 40
# baseline (speedup 1.0000x reference)
"""BioWaveKAN fused kernel for 8 Trainium2 NeuronCores.

y = wavelet(u) @ (pi^-1/4 * Ww).T + u @ (0.3 * s * Wb).T
out = BatchNorm1d(y)  (training-mode batch stats, all-reduced across cores)

Host sends u = (x - translate)/clamp(scale, 1e-3) directly; the per-feature
scale folds into the base weights and the translate constant is cancelled
exactly by BN mean subtraction.  All wavelet affines on device are then
constant-scalar big-chunk ops:
  t = u*(3/2pi) + 1/4;  r = t - round(t)  (magic-number round, DVE)
  cos(3u) = sin(2pi r)  (ACT Sin);  gauss = exp(-0.5 u^2)  (ACT Square+Exp)

Sharding: data-parallel over batch (8 x 512 rows).  Single-pass PSUM
accumulation per o-quarter (u-half then wavelet-half; group order
h0q0 h0q1 h1q0 h1q1 h0q2 h1q2 h0q3 h1q3 keeps exactly 8 PSUM banks live
and spreads the stats exchanges).  BN stats cross-core via tiny
AllGathers + local tree-reduce (~4x faster than AllReduce here); a dummy
gather at t~0 absorbs the first-collective warmup under the matmuls.
Drains: DVE STT with sum accumulator + ACT Square with sumsq accumulator.
Finalize for q0-q2 on GpSimd (cannot block DVE drains), q3 on DVE.
"""
import math

import numpy as np

from concourse import bacc
import concourse.tile as tile
import concourse.mybir as mybir
from concourse.bass_utils import run_bass_kernel_spmd

F32 = mybir.dt.float32
F16 = mybir.dt.float16
AF = mybir.ActivationFunctionType
OP = mybir.AluOpType

B = 4096          # batch
D = 2048          # in_dim == out_dim
NCORES = 8
BS = B // NCORES  # batch shard per core (512)
NIT = D // 128    # i-tiles (16)
NOT = D // 128    # o-tiles (16)
NQ = 4            # quarters of o-tiles
NC = 4            # wavelet chunks (4 i-tiles each)
BN_EPS = 1e-5
TWO_PI = 2.0 * math.pi
A3 = 3.0 / TWO_PI   # t = A3*u + B3 ; sin(2pi t) = cos(3u)
B3 = 0.25
MAGIC = 1.5 * 2.0 ** 23

_CACHE = {}


def _build_nc():
    nc = bacc.Bacc()

    # u laid out as [128p, NIT*BS]: row p holds, for each kt, u[kt*128+p, b]
    u_d = nc.dram_tensor("u", (128, NIT * BS), F16, kind="ExternalInput")
    # weights as [8 groups (h*4+q) x 128p, NIT*512]: row (g,p) holds, for
    # each kt, W'[q*512+oc, kt*128+p] for oc in 0..512
    w_d = nc.dram_tensor("w", (8 * 128, NIT * 512), F16, kind="ExternalInput")
    cst_d = nc.dram_tensor("cst", (128, 2 * NOT), F32, kind="ExternalInput")

    yT_d = nc.dram_tensor("yT", (128, NOT * BS), F16, kind="ExternalOutput")

    # stats column layout: quarter q holds cols [8q, 8q+8):
    #   [8q + ml]     = sum(y)   for o-tile m = 4q + ml
    #   [8q + 4 + ml] = sum(y^2)
    with tile.TileContext(nc) as tc:
        with (
            tc.tile_pool(name="big", bufs=1) as big,
            tc.tile_pool(name="small", bufs=1) as small,
            tc.tile_pool(name="wk", bufs=4) as wk,
            tc.tile_pool(name="wqh", bufs=3) as wqh,
            tc.tile_pool(name="wq", bufs=3) as wq,
            tc.tile_pool(name="scr", bufs=5) as scr,
            tc.tile_pool(name="escr", bufs=2) as escr,
            tc.tile_pool(name="dscr", bufs=3) as dscr,
            tc.tile_pool(name="ps", bufs=8, space="PSUM") as ps,
            tc.tile_pool(name="dram", bufs=1, space="DRAM") as dram,
        ):
            rhs = big.tile([128, 2 * NIT, BS], F16)   # 0..15 u, 16..31 wavelet
            y16 = big.tile([128, NOT, BS], F16)
            nc.gpsimd.memset(y16[:], 0.0)

            # ---- PE pre-warm: dep-free junk matmuls run during the DMA wait
            # so HAM un-throttles (1.2->2.4GHz) before real matmuls arrive
            jmm = small.tile([128, BS], F16)
            nc.vector.memset(jmm[:], 0.0)
            jps = ps.tile([128, BS], F32, tag="ps", name="jps")
            for _ in range(9):
                nc.tensor.matmul(jps[:], jmm[:, 0:128], jmm[:],
                                 start=True, stop=True)

            # ---- DMA issue order: first kt-tiles split small across DMA
            # queues (parallel arrival), then the rest in bigger chunks ----
            w00k = []
            nc.sync.dma_start(rhs[:, 0, :], u_d[:, 0:BS])
            wt0 = wk.tile([128, 512], F16, tag="wk", name="w00k_0")
            nc.sync.dma_start(wt0[:], w_d[0:128, 0:512])
            w00k.append(wt0)
            nc.sync.dma_start(rhs[:, 1, :], u_d[:, BS:2 * BS])
            for kt in range(1, 4):
                wt = wk.tile([128, 512], F16, tag="wk", name=f"w00k_{kt}")
                nc.sync.dma_start(wt[:], w_d[0:128, kt * 512:(kt + 1) * 512])
                w00k.append(wt)
            nc.sync.dma_start(rhs[:, 2:4, :], u_d[:, 2 * BS:4 * BS])
            w00 = []
            for s in range(1, 4):
                wt = wqh.tile([128, 4 * 512], F16, tag="wqh", name=f"w00_{s}")
                nc.sync.dma_start(wt[:], w_d[0:128, s * 4 * 512:(s + 1) * 4 * 512])
                w00.append(wt)
                nc.sync.dma_start(rhs[:, 4 * s:4 * (s + 1), :],
                                  u_d[:, 4 * s * BS:4 * (s + 1) * BS])

            cstt = small.tile([128, 2 * NOT], F32)
            nc.sync.dma_start(cstt[:], cst_d[:])
            gmt = cstt[:, 0:NOT]
            btt = cstt[:, NOT:2 * NOT]

            # group execution order (h, q); single-pass accumulation per q
            GSEQ = [(0, 0), (0, 1), (1, 0), (1, 1), (0, 2), (1, 2), (0, 3),
                    (1, 3)]
            wtiles = {}
            for (h, q) in GSEQ[1:]:
                g = h * 4 + q
                wt = wq.tile([128, NIT * 512], F16, tag="wq", name=f"w_{g}")
                nc.sync.dma_start(wt[:], w_d[g * 128:(g + 1) * 128, :])
                wtiles[g] = wt

            # ---- wavelet phase A: sin tiles (trig table set) ----
            zbt = small.tile([128, 1], F32)
            nc.vector.memset(zbt[:], 0.0)
            epst = small.tile([128, 1], F32)
            nc.vector.memset(epst[:], BN_EPS)

            # preload the sin table set immediately (dep-free junk op)
            junks = small.tile([128, 1], F32)
            nc.scalar.activation(junks[:], zbt[:], AF.Sin, bias=zbt[:],
                                 scale=TWO_PI)

            rts = []
            for c in range(NC):
                uf = rhs[:, 4 * c:4 * c + 4, :]
                tt = scr.tile([128, 4, BS], F32, tag="scr", name=f"t_{c}")
                nc.vector.tensor_scalar(out=tt[:], in0=uf,
                                        scalar1=A3, scalar2=B3,
                                        op0=OP.mult, op1=OP.add)
                vt = scr.tile([128, 4, BS], F32, tag="scr", name=f"v_{c}")
                nc.vector.tensor_scalar(out=vt[:], in0=tt[:],
                                        scalar1=MAGIC, scalar2=MAGIC,
                                        op0=OP.add, op1=OP.subtract)
                rt = scr.tile([128, 4, BS], F32, tag="scr", name=f"r_{c}")
                nc.vector.tensor_tensor(rt[:], tt[:], vt[:], op=OP.subtract)
                rts.append(rt)
            for c in range(NC):
                nc.scalar.activation(rhs[:, 16 + 4 * c:16 + 4 * c + 4, :],
                                     rts[c][:], AF.Sin, bias=zbt[:],
                                     scale=TWO_PI)

            # ---- wavelet phase B: gaussian (exp table set) ----
            # bias tile derived from the last sin output: forces all phase-B
            # ACT ops after all Sins (one exp-set table load, no thrash)
            zbt2 = small.tile([128, 1], F32)
            nc.gpsimd.tensor_single_scalar(out=zbt2[:], in_=rhs[:, 31, 0:1],
                                           scalar=0.0, op=OP.mult)
            ets = []
            for c in range(NC):
                uf = rhs[:, 4 * c:4 * c + 4, :]
                qt = scr.tile([128, 4, BS], F32, tag="scr", name=f"qq_{c}")
                nc.scalar.activation(qt[:], uf, AF.Square, bias=zbt2[:])
                et = escr.tile([128, 4, BS], F16, tag="escr", name=f"e_{c}")
                nc.scalar.activation(et[:], qt[:], AF.Exp, bias=zbt[:],
                                     scale=-0.5)
                ets.append(et)
                nc.vector.tensor_tensor(rhs[:, 16 + 4 * c:16 + 4 * c + 4, :],
                                        rhs[:, 16 + 4 * c:16 + 4 * c + 4, :],
                                        et[:], op=OP.mult)

            # prefetch the sqrt table set (Square + Sqrt share it) right
            # after phase B; input depends on the last exp tile so the
            # scheduler cannot hoist this ahead of the sin/exp phases
            junk2 = small.tile([128, 1], F32)
            nc.scalar.activation(junk2[:], ets[-1][:, 0, 0:1], AF.Sqrt,
                                 bias=epst[:])

            # ---- matmuls + fused drains + per-quarter stats AllReduce ----
            stats = small.tile([128, 2 * NOT], F32)
            ab = small.tile([128, 2 * NOT], F32)   # A cols 0..15, B cols 16..31

            # stats exchange: AllGather raw per-core sums (half the latency of
            # AllReduce) + local tree-reduce.  A dummy gather at t~0 absorbs
            # the first-collective warmup cost under the matmul stream.
            CCW = {"d": 2, "01": 16, "2": 8, "3": 8}
            ibs, obs = {}, {}
            for k, w in CCW.items():
                ibs[k] = dram.tile([128, w], F32, name=f"ib{k}")
                obs[k] = dram.tile([NCORES, 128, w], F32, name=f"ob{k}")

            dumt = small.tile([128, 2], F32)
            nc.gpsimd.memset(dumt[:], 1.0)
            nc.sync.dma_start(ibs["d"][:], dumt[:])
            nc.gpsimd.collective_compute(
                "AllGather", OP.bypass,
                replica_groups=[list(range(NCORES))],
                ins=[ibs["d"].opt()], outs=[obs["d"].opt()])

            qpsums = {}
            for (h, q) in GSEQ:
                if h == 0:
                    qpsums[q] = [ps.tile([128, BS], F32, tag="ps",
                                         name=f"pst_{q}_{_pi}")
                                 for _pi in range(4)]
                psums = qpsums[q]
                # h==1 runs ml-outer so each o-tile's psum stops early and
                # its drain staggers ahead of the group end
                if h == 0:
                    loop = [(kt, ml) for kt in range(NIT) for ml in range(4)]
                else:
                    loop = [(kt, ml) for ml in range(4) for kt in range(NIT)]
                for kt, ml in loop:
                    if (h, q) == (0, 0):
                        if kt < 4:
                            wsl = w00k[kt][:]
                        else:
                            wsl = w00[kt // 4 - 1][:, (kt % 4) * 512:
                                                   (kt % 4 + 1) * 512]
                    else:
                        wsl = wtiles[h * 4 + q][:, kt * 512:(kt + 1) * 512]
                    nc.tensor.matmul(
                        psums[ml][:],
                        wsl[:, ml * 128:(ml + 1) * 128],
                        rhs[:, h * NIT + kt, :],
                        start=(h == 0 and kt == 0),
                        stop=(h == 1 and kt == NIT - 1))
                if h == 1:
                    for ml in range(4):
                        m = q * 4 + ml
                        nc.vector.scalar_tensor_tensor(
                            out=y16[:, m, :], in0=psums[ml][:],
                            scalar=1.0, in1=y16[:, m, :],
                            op0=OP.mult, op1=OP.add,
                            accum_out=stats[:, 8 * q + ml:8 * q + ml + 1])
                        dsc = dscr.tile([128, BS], F32, tag="dscr",
                                        name=f"dsc_{m}")
                        nc.scalar.activation(
                            dsc[:], y16[:, m, :], AF.Square,
                            bias=zbt[:],
                            accum_out=stats[:, 8 * q + 4 + ml:
                                            8 * q + 5 + ml])
                    key = {1: "01", 2: "2", 3: "3"}.get(q)
                    if key is not None:
                        lo = 0 if key == "01" else 8 * q
                        nc.sync.dma_start(ibs[key][:],
                                          stats[:, lo:lo + CCW[key]])
                        nc.gpsimd.collective_compute(
                            "AllGather", OP.bypass,
                            replica_groups=[list(range(NCORES))],
                            ins=[ibs[key].opt()], outs=[obs[key].opt()])

            # ---- per-quarter finalize + normalize + store ----
            gats = {}
            for k in ("01", "2", "3"):
                w = CCW[k]
                gat = small.tile([128, NCORES, w], F32, name=f"gat{k}")
                nc.sync.dma_start(gat[:], obs[k][:].rearrange("r p c -> p r c"))
                eng = nc.vector if k == "3" else nc.gpsimd
                eng.tensor_tensor(gat[:, 0:4, :], gat[:, 0:4, :],
                                  gat[:, 4:8, :], op=OP.add)
                eng.tensor_tensor(gat[:, 0:2, :], gat[:, 0:2, :],
                                  gat[:, 2:4, :], op=OP.add)
                eng.tensor_tensor(gat[:, 0:1, :], gat[:, 0:1, :],
                                  gat[:, 1:2, :], op=OP.add)
                gats[k] = gat
            redcols = {0: (gats["01"], 0), 1: (gats["01"], 8),
                       2: (gats["2"], 0), 3: (gats["3"], 0)}
            for q in range(NQ):
                # q0..q2 finalize on GpSimd (cannot block DVE drains);
                # q3 on DVE (tail, nothing scheduled after it)
                eng = nc.vector if q == 3 else nc.gpsimd
                gt, off = redcols[q]
                mean = small.tile([128, 4], F32, name=f"mean{q}")
                eng.tensor_single_scalar(
                    out=mean[:], in_=gt[:, 0, off:off + 4],
                    scalar=1.0 / B, op=OP.mult)
                msq = small.tile([128, 4], F32, name=f"msq{q}")
                eng.tensor_single_scalar(
                    out=msq[:], in_=gt[:, 0, off + 4:off + 8],
                    scalar=1.0 / B, op=OP.mult)
                var = small.tile([128, 4], F32, name=f"var{q}")
                eng.tensor_tensor(var[:], mean[:], mean[:], op=OP.mult)
                eng.tensor_tensor(var[:], msq[:], var[:], op=OP.subtract)
                stdt = small.tile([128, 4], F32, name=f"std{q}")
                nc.scalar.activation(stdt[:], var[:], AF.Sqrt, bias=epst[:])
                rstd = small.tile([128, 4], F32, name=f"rstd{q}")
                nc.vector.reciprocal(out=rstd[:], in_=stdt[:])
                acols = ab[:, 4 * q:4 * q + 4]
                bcols = ab[:, 16 + 4 * q:16 + 4 * q + 4]
                eng.tensor_tensor(acols, gmt[:, 4 * q:4 * q + 4],
                                  rstd[:], op=OP.mult)
                eng.tensor_tensor(bcols, mean[:], acols, op=OP.mult)
                eng.tensor_tensor(bcols, btt[:, 4 * q:4 * q + 4],
                                  bcols, op=OP.subtract)

                for ml in range(4):
                    m = q * 4 + ml
                    nc.vector.tensor_scalar(
                        out=y16[:, m, :], in0=y16[:, m, :],
                        scalar1=ab[:, m:m + 1],
                        scalar2=ab[:, 16 + m:17 + m],
                        op0=OP.mult, op1=OP.add)
                nc.sync.dma_start(yT_d[:, q * 4 * BS:(q + 1) * 4 * BS],
                                  y16[:, q * 4:(q + 1) * 4, :])

    nc.compile()
    return nc


def _get_nc():
    if "nc" not in _CACHE:
        _CACHE["nc"] = _build_nc()
    return _CACHE["nc"]


def _fold(v):
    """(1, D) or (D,) feature vector -> (128, NOT) column-per-o-tile layout."""
    return np.ascontiguousarray(v.reshape(NOT, 128).T).astype(np.float32)


def kernel(x, scale, translate, wave_weight, base_weight, gamma, beta):
    x = np.asarray(x, dtype=np.float32)
    scale = np.asarray(scale, dtype=np.float32).reshape(1, D)
    translate = np.asarray(translate, dtype=np.float32).reshape(1, D)
    wave_weight = np.asarray(wave_weight, dtype=np.float32)
    base_weight = np.asarray(base_weight, dtype=np.float32)
    gamma = np.asarray(gamma, dtype=np.float32).reshape(D)
    beta = np.asarray(beta, dtype=np.float32).reshape(D)

    inv_s = 1.0 / np.maximum(scale, 1e-3)                     # (1, D)
    u = (x - translate) * inv_s                               # (B, D)

    # base path absorbs the feature scale; the translate-induced constant
    # shift per output feature is cancelled exactly by BN mean subtraction:
    # base_out = x @ Wb.T = (u*s + t) @ Wb.T -> u @ (s*Wb).T + const
    wb = 0.3 * base_weight * np.maximum(scale, 1e-3)          # (O, I) * (1, I)
    ww = (math.pi ** -0.25) * wave_weight

    # weight groups: g = h*4 + q; block [128, NIT*512] where row p holds,
    # for each kt, W'[q*512 + oc, kt*128 + p]
    def _wfold(W):
        # W: (O, I) -> (4 quarters, 128 p, NIT kt, 512 oc)
        Wt = W.reshape(NQ, 512, NIT, 128)        # [q, oc, kt, p]
        return Wt.transpose(0, 3, 2, 1)          # [q, p, kt, oc]

    wcat = np.concatenate([_wfold(wb), _wfold(ww)], axis=0)   # [8, 128, NIT, 512]
    wcat = np.ascontiguousarray(
        wcat.reshape(8 * 128, NIT * 512).astype(np.float16))

    # u tiles: [128 p, NIT kt, BS] per core, row-contiguous
    uT = u.T.astype(np.float16)                               # (D, B)
    uT = uT.reshape(NIT, 128, B).transpose(1, 0, 2)           # [p, kt, B]

    cst = np.concatenate([_fold(gamma), _fold(beta)], axis=1)
    common = dict(w=wcat, cst=np.ascontiguousarray(cst))
    in_maps = [
        dict(u=np.ascontiguousarray(
            uT[:, :, c * BS:(c + 1) * BS].reshape(128, NIT * BS)), **common)
        for c in range(NCORES)
    ]

    nc = _get_nc()
    res = run_bass_kernel_spmd(nc, in_maps, core_ids=list(range(NCORES)),
                               **_CACHE.pop("run_kwargs", {}))
    _CACHE["last_res"] = res
    # yT: [128, NOT*BS] per core; feature o = m*128 + p
    outs = []
    for c in range(NCORES):
        yT = res.results[c]["yT"].reshape(128, NOT, BS).transpose(1, 0, 2)
        outs.append(yT.reshape(D, BS))
    full = np.concatenate(outs, axis=1)                       # (D, B)
    return np.ascontiguousarray(full.T.astype(np.float32))
